# revision 15
# baseline (speedup 1.0000x reference)
"""W8A8 quantized linear (BitBLAS-style) on 8 Trainium2 NeuronCores.

Computation (matches the fp reference exactly up to fp32 rounding):
  absmax  = max|x|                     (launch 1: token-sharded across cores)
  x_q     = round(x * 127/(absmax+1e-8))  as integer-valued bf16 (exact for |v|<=127)
  out     = (x_q @ qweight.T) * (absmax/127 * weight_scale)   (launch 2)

Sharding: column-parallel — qweight/weight_scale split along out_features
across the 8 cores; x replicated; no cross-core reduction needed.

The int8 GEMM is run as bf16 x bf16 with fp32 PSUM accumulation: int8
values are exact in bf16, products <= 127^2 and partial sums << 2^24 are
exact in fp32, so results are bit-identical to an int32 GEMM for this data.
"""
import sys

sys.path.insert(0, "/opt/trn_rl_repo")

import numpy as np

import concourse.bass as bass
import concourse.mybir as mybir
from concourse import tile
from concourse.bass_utils import run_bass_kernel_spmd
from concourse.vector_clock import ScopedClock

F32 = mybir.dt.float32
BF16 = mybir.dt.bfloat16
I8 = mybir.dt.int8

B, S, K = 2, 2048, 4096
T = B * S          # 4096 tokens
N = 4096           # out features
NCORES = 8
NSH = N // NCORES  # 512 out features per core
KT = K // 128      # 32 k-tiles
NT = NSH // 128    # 4 n-tiles per core
TC = 512           # token chunk (matmul moving free dim)
NCH = T // TC      # 8 chunks
QG = 8             # k-tiles per quantization group
NQG = KT // QG     # 4 groups per chunk

MAGIC = np.float32(12582912.0)  # 1.5 * 2^23: fp32 round-half-even to integer

# ---------------------------------------------------------------------------
# The walrus build in this container only accepts ONE sync-wait command per
# Drain instruction; Tile's final drain attaches one wait per active proc.
# Split the excess waits across extra drains on the sync engine.
_MAX_DRAIN_WAITS = 1


def _patched_drain_and_barrier(self, tick_clock, wait_clock):
    import bass_rust as _br

    nc = self.nc
    drain_inst = nc.sync.drain()
    wait_clock.add_sem_waits(
        drain_inst.ins, ScopedClock({None: tick_clock.global_clock})
    )
    waits = list(drain_inst.ins.sync_info.on_wait or [])
    if len(waits) > _MAX_DRAIN_WAITS:
        drain_inst.ins.sync_info.on_wait = waits[:_MAX_DRAIN_WAITS]
        rest = waits[_MAX_DRAIN_WAITS:]
        for i in range(0, len(rest), _MAX_DRAIN_WAITS):
            extra = nc.sync.drain()
            extra.ins.sync_info = _br.SyncInfo(
                on_wait=rest[i : i + _MAX_DRAIN_WAITS], on_update=[]
            )

    nc.all_engine_barrier()
    assert self.sems is not None
    popped = nc._tile_sem_poison_stack.pop()
    assert popped is self._sem_poison
    nc.clear_and_free_semaphores(list(self.sems.allocated().values()))
    nc.all_engine_barrier()


tile.TileContext._drain_and_barrier = _patched_drain_and_barrier

_waitsplit_seq = [0]


def _split_excess_waits(nc, limit=1):
    """Walrus here accepts at most `limit` sync waits per instruction.
    Hoist excess waits onto standalone EventSemaphore instructions spliced
    immediately before the over-subscribed instruction on the same engine
    (same basic block, so per-engine program order is preserved)."""
    import bass_rust as _br

    for f in nc.m.functions:
        for blk in f.blocks:
            il = blk.instructions
            if not any(
                getattr(inst, "sync_info", None)
                and inst.sync_info.on_wait
                and len(inst.sync_info.on_wait) > limit
                for inst in il
            ):
                continue
            new_list = []
            for inst in il:
                si = getattr(inst, "sync_info", None)
                waits = list(si.on_wait) if si and si.on_wait else []
                if len(waits) > limit:
                    for j in range(limit, len(waits), limit):
                        carrier = mybir.InstEventSemaphore(
                            name=f"waitsplit_{_waitsplit_seq[0]}",
                            opcode="EventSemaphore",
                            engine=inst.engine,
                            sync_info=_br.SyncInfo(
                                on_wait=waits[j : j + limit], on_update=[]
                            ),
                        )
                        _waitsplit_seq[0] += 1
                        new_list.append(carrier)
                    si.on_wait = waits[:limit]
                new_list.append(inst)
            blk.instructions[:] = new_list


# ---------------------------------------------------------------------------

_NC_CACHE = {}


def _absmax_nc():
    """Per-core: xs [K/8=512, T] f32 -> amax [128, 1] f32 (per-partition max|.|).

    (Tile-based: raw-Bass variant left dirty post-barrier semaphore state
    that corrupted the next NEFF on the same core.)"""
    if "absmax" in _NC_CACHE:
        return _NC_CACHE["absmax"]
    nc = bass.Bass(name="w8a8_absmax")
    xs = nc.declare_dram_parameter("xs", [K // NCORES, T], F32, isOutput=False)
    amax = nc.declare_dram_parameter("amax", [128, 1], F32, isOutput=True)
    xs_r = xs.rearrange("(a p) t -> p a t", p=128)  # [128, 4, T]
    with tile.TileContext(nc) as tc:
        with (
            tc.tile_pool(name="xin", bufs=4) as xin,
            tc.tile_pool(name="m", bufs=1) as mpool,
        ):
            part = mpool.tile([128, 5], F32)
            for i in range(4):
                xt = xin.tile([128, T], F32)
                nc.sync.dma_start(xt[:], xs_r[:, i, :])
                nc.vector.tensor_reduce(
                    part[:, i : i + 1],
                    xt[:],
                    axis=mybir.AxisListType.X,
                    op=mybir.AluOpType.max,
                    apply_absolute_value=True,
                )
            nc.vector.tensor_reduce(
                part[:, 4:5],
                part[:, 0:4],
                axis=mybir.AxisListType.X,
                op=mybir.AluOpType.max,
                apply_absolute_value=True,
            )
            nc.sync.dma_start(amax[:], part[:, 4:5])
    _split_excess_waits(nc)
    _NC_CACHE["absmax"] = nc
    return nc


def _main_nc():
    """Per-core fused quantize + GEMM + dequant.

    Inputs : xT [K, T] f32 (replicated, K-major), wT [K, NSH] int8,
             qs [128,1] f32 (127/(absmax+1e-8)), cs [128, NT] f32
             (act_scale * weight_scale arranged [partition, n-tile]).
    Output : out [NSH, T] f32 (the core's out-feature shard, n-major).
    """
    if "main" in _NC_CACHE:
        return _NC_CACHE["main"]
    nc = bass.Bass(name="w8a8_main")
    xT = nc.declare_dram_parameter("xT", [K, T], F32, isOutput=False)
    wT = nc.declare_dram_parameter("wT", [K, NSH], I8, isOutput=False)
    qs = nc.declare_dram_parameter("qs", [128, 1], F32, isOutput=False)
    cs = nc.declare_dram_parameter("cs", [128, NT], F32, isOutput=False)
    out = nc.declare_dram_parameter("out", [NSH, T], F32, isOutput=True)

    xT_r = xT.rearrange("(a p) t -> p a t", p=128)    # [128, KT, T]
    wT_r = wT.rearrange("(a p) n -> p a n", p=128)    # [128, KT, NSH]
    out_r = out.rearrange("(a p) t -> p a t", p=128)  # [128, NT, T]

    with tile.TileContext(nc) as tc:
        with (
            tc.tile_pool(name="const", bufs=1) as cpool,
            tc.tile_pool(name="wbf", bufs=1) as wpool,
            tc.tile_pool(name="xstage", bufs=2) as xspool,
            tc.tile_pool(name="xtmp", bufs=2) as xtpool,
            tc.tile_pool(name="xq", bufs=2) as xqpool,
            tc.tile_pool(name="psum", bufs=4, space="PSUM") as pspool,
            tc.tile_pool(name="ostage", bufs=4) as opool,
        ):
            qs_t = cpool.tile([128, 1], F32)
            nc.sync.dma_start(qs_t[:], qs[:])
            cs_t = cpool.tile([128, NT], F32)
            nc.gpsimd.dma_start(cs_t[:], cs[:])

            # PE warm-up: a few matmuls on (uninitialized) SBUF keep the PE
            # busy during the input DMAs so HAM un-throttles the clock to
            # 2.4 GHz before the first real matmul. Results are discarded.
            wbf = wpool.tile([128, KT, NSH], BF16)
            warm_ps = pspool.tile([128, TC], F32, tag="ps")
            for _ in range(14):
                nc.tensor.matmul(
                    warm_ps[:],
                    wbf[:, 0, 0:128],
                    wbf[:, 1, 0:TC],
                    start=True,
                    stop=True,
                    skip_group_check=True,
                )

            # Weights: DMA int8 -> SBUF (triggered from the GpSimd queue so
            # the Sync queue issues the latency-critical x chunk loads),
            # widen to bf16 (exact) on ACT, per k-group so the first
            # matmuls' weights are ready early. Staged in an xq-pool slot
            # (16 KiB < the 32 KiB xq slot): costs no extra SBUF.
            ws = xqpool.tile([128, KT, NSH], I8, tag="xq")
            for g in range(NQG):
                sl = slice(g * QG, (g + 1) * QG)
                nc.gpsimd.dma_start(ws[:, sl, :], wT_r[:, sl, :])
                nc.scalar.activation(
                    wbf[:, sl, :],
                    ws[:, sl, :],
                    mybir.ActivationFunctionType.Copy,
                )

            for ch in range(NCH):
                t0 = ch * TC
                # --- quantize this token chunk: xq = round(x * qs) in bf16 ---
                xq = xqpool.tile([128, KT, TC], BF16)
                for g in range(NQG):
                    xs = xspool.tile([128, QG, TC], F32)
                    nc.sync.dma_start(
                        xs[:], xT_r[:, g * QG : (g + 1) * QG, t0 : t0 + TC]
                    )
                    tmp = xtpool.tile([128, QG, TC], F32)
                    # (x * qs) rounds to fp32 between the two ALU slices,
                    # then +MAGIC rounds half-even to integer: identical to
                    # jnp.round(x * scale_inv).
                    nc.vector.tensor_scalar(
                        tmp[:],
                        xs[:],
                        qs_t[:, 0:1],
                        float(MAGIC),
                        op0=mybir.AluOpType.mult,
                        op1=mybir.AluOpType.add,
                    )
                    nc.vector.tensor_scalar_add(
                        xq[:, g * QG : (g + 1) * QG, :], tmp[:], -float(MAGIC)
                    )
                # --- GEMM: psum[n128, TC] = sum_k wT[k,n].T @ xq[k,t] ---
                for nt in range(NT):
                    ps = pspool.tile([128, TC], F32, tag="ps")
                    for kt in range(KT):
                        nc.tensor.matmul(
                            ps[:],
                            wbf[:, kt, nt * 128 : (nt + 1) * 128],
                            xq[:, kt, :],
                            start=(kt == 0),
                            stop=(kt == KT - 1),
                        )
                    ot = opool.tile([128, TC], F32)
                    nc.scalar.activation(
                        ot[:],
                        ps[:],
                        mybir.ActivationFunctionType.Copy,
                        scale=cs_t[:, nt : nt + 1],
                    )
                    nc.sync.dma_start(out_r[:, nt, t0 : t0 + TC], ot[:])
    _split_excess_waits(nc)
    _NC_CACHE["main"] = nc
    return nc


def kernel(x, qweight, weight_scale):
    x = np.asarray(x)
    orig_dtype = x.dtype
    x2 = np.ascontiguousarray(x, dtype=np.float32).reshape(T, K)
    qw = np.asarray(qweight)
    if qw.dtype != np.int8:
        qw = qw.astype(np.int8)
    ws = np.asarray(weight_scale, dtype=np.float32)

    xT = np.ascontiguousarray(x2.T)  # [K, T]
    core_ids = list(range(NCORES))

    # --- launch 1: global absmax (each core reduces a K-shard of xT) ---
    ksh = K // NCORES
    res1 = run_bass_kernel_spmd(
        _absmax_nc(),
        [{"xs": xT[c * ksh : (c + 1) * ksh, :]} for c in core_ids],
        core_ids=core_ids,
    )
    absmax = np.float32(
        max(np.float32(r["amax"].max()) for r in res1.results)
    )
    scale_inv = np.float32(127.0) / (absmax + np.float32(1e-8))
    act_scale = absmax / np.float32(127.0)

    # --- launch 2: quantize + GEMM + dequant, column-parallel ---
    qs_arr = np.full((128, 1), scale_inv, dtype=np.float32)
    in_maps = []
    for c in core_ids:
        w_shard = qw[c * NSH : (c + 1) * NSH, :]  # [NSH, K] int8
        wT_shard = np.ascontiguousarray(w_shard.T)  # [K, NSH]
        cs_arr = np.ascontiguousarray(
            (act_scale * ws[c * NSH : (c + 1) * NSH]).reshape(NT, 128).T
        )  # [128, NT]
        in_maps.append(
            {"xT": xT, "wT": wT_shard, "qs": qs_arr, "cs": cs_arr}
        )
    res2 = run_bass_kernel_spmd(_main_nc(), in_maps, core_ids=core_ids)

    outT = np.concatenate(
        [res2.results[c]["out"] for c in core_ids], axis=0
    )  # [N, T]
    return (
        np.ascontiguousarray(outT.T)
        .reshape(B, S, N)
        .astype(orig_dtype, copy=False)
    )


# revision 19
# speedup vs baseline: 1.0054x; 1.0054x over previous
"""W8A8 quantized linear (BitBLAS-style) on 8 Trainium2 NeuronCores.

Computation (matches the fp reference exactly up to fp32 rounding):
  absmax  = max|x|                     (launch 1: token-sharded across cores)
  x_q     = round(x * 127/(absmax+1e-8))  as integer-valued bf16 (exact for |v|<=127)
  out     = (x_q @ qweight.T) * (absmax/127 * weight_scale)   (launch 2)

Sharding: column-parallel — qweight/weight_scale split along out_features
across the 8 cores; x replicated; no cross-core reduction needed.

The int8 GEMM is run as bf16 x bf16 with fp32 PSUM accumulation: int8
values are exact in bf16, products <= 127^2 and partial sums << 2^24 are
exact in fp32, so results are bit-identical to an int32 GEMM for this data.
"""
import sys

sys.path.insert(0, "/opt/trn_rl_repo")

import numpy as np

import concourse.bass as bass
import concourse.mybir as mybir
from concourse import tile
from concourse.bass_utils import run_bass_kernel_spmd
from concourse.vector_clock import ScopedClock

F32 = mybir.dt.float32
BF16 = mybir.dt.bfloat16
I8 = mybir.dt.int8

B, S, K = 2, 2048, 4096
T = B * S          # 4096 tokens
N = 4096           # out features
NCORES = 8
NSH = N // NCORES  # 512 out features per core
KT = K // 128      # 32 k-tiles
NT = NSH // 128    # 4 n-tiles per core
TC = 512           # token chunk (matmul moving free dim)
NCH = T // TC      # 8 chunks
QG = 8             # k-tiles per quantization group
NQG = KT // QG     # 4 groups per chunk

MAGIC = np.float32(12582912.0)  # 1.5 * 2^23: fp32 round-half-even to integer

# ---------------------------------------------------------------------------
# The walrus build in this container only accepts ONE sync-wait command per
# Drain instruction; Tile's final drain attaches one wait per active proc.
# Split the excess waits across extra drains on the sync engine.
_MAX_DRAIN_WAITS = 1


def _patched_drain_and_barrier(self, tick_clock, wait_clock):
    import bass_rust as _br

    nc = self.nc
    drain_inst = nc.sync.drain()
    wait_clock.add_sem_waits(
        drain_inst.ins, ScopedClock({None: tick_clock.global_clock})
    )
    waits = list(drain_inst.ins.sync_info.on_wait or [])
    if len(waits) > _MAX_DRAIN_WAITS:
        drain_inst.ins.sync_info.on_wait = waits[:_MAX_DRAIN_WAITS]
        rest = waits[_MAX_DRAIN_WAITS:]
        for i in range(0, len(rest), _MAX_DRAIN_WAITS):
            extra = nc.sync.drain()
            extra.ins.sync_info = _br.SyncInfo(
                on_wait=rest[i : i + _MAX_DRAIN_WAITS], on_update=[]
            )

    nc.all_engine_barrier()
    assert self.sems is not None
    popped = nc._tile_sem_poison_stack.pop()
    assert popped is self._sem_poison
    nc.clear_and_free_semaphores(list(self.sems.allocated().values()))
    nc.all_engine_barrier()


tile.TileContext._drain_and_barrier = _patched_drain_and_barrier

_waitsplit_seq = [0]


def _split_excess_waits(nc, limit=1):
    """Walrus here accepts at most `limit` sync waits per instruction.
    Hoist excess waits onto standalone EventSemaphore instructions spliced
    immediately before the over-subscribed instruction on the same engine
    (same basic block, so per-engine program order is preserved)."""
    import bass_rust as _br

    for f in nc.m.functions:
        for blk in f.blocks:
            il = blk.instructions
            if not any(
                getattr(inst, "sync_info", None)
                and inst.sync_info.on_wait
                and len(inst.sync_info.on_wait) > limit
                for inst in il
            ):
                continue
            new_list = []
            for inst in il:
                si = getattr(inst, "sync_info", None)
                waits = list(si.on_wait) if si and si.on_wait else []
                if len(waits) > limit:
                    for j in range(limit, len(waits), limit):
                        carrier = mybir.InstEventSemaphore(
                            name=f"waitsplit_{_waitsplit_seq[0]}",
                            opcode="EventSemaphore",
                            engine=inst.engine,
                            sync_info=_br.SyncInfo(
                                on_wait=waits[j : j + limit], on_update=[]
                            ),
                        )
                        _waitsplit_seq[0] += 1
                        new_list.append(carrier)
                    si.on_wait = waits[:limit]
                new_list.append(inst)
            blk.instructions[:] = new_list


# ---------------------------------------------------------------------------

_NC_CACHE = {}


def _absmax_nc():
    """Per-core: xs [K/8=512, T] f32 -> amax [128, 1] f32 (per-partition max|.|).

    (Tile-based: raw-Bass variant left dirty post-barrier semaphore state
    that corrupted the next NEFF on the same core.)"""
    if "absmax" in _NC_CACHE:
        return _NC_CACHE["absmax"]
    nc = bass.Bass(name="w8a8_absmax")
    xs = nc.declare_dram_parameter("xs", [K // NCORES, T], F32, isOutput=False)
    amax = nc.declare_dram_parameter("amax", [128, 1], F32, isOutput=True)
    xs_r = xs.rearrange("(a p) t -> p a t", p=128)  # [128, 4, T]
    with tile.TileContext(nc) as tc:
        with (
            tc.tile_pool(name="xin", bufs=4) as xin,
            tc.tile_pool(name="m", bufs=1) as mpool,
        ):
            part = mpool.tile([128, 5], F32)
            for i in range(4):
                xt = xin.tile([128, T], F32)
                nc.sync.dma_start(xt[:], xs_r[:, i, :])
                nc.vector.tensor_reduce(
                    part[:, i : i + 1],
                    xt[:],
                    axis=mybir.AxisListType.X,
                    op=mybir.AluOpType.max,
                    apply_absolute_value=True,
                )
            nc.vector.tensor_reduce(
                part[:, 4:5],
                part[:, 0:4],
                axis=mybir.AxisListType.X,
                op=mybir.AluOpType.max,
                apply_absolute_value=True,
            )
            nc.sync.dma_start(amax[:], part[:, 4:5])
    _split_excess_waits(nc)
    _NC_CACHE["absmax"] = nc
    return nc


def _main_nc():
    """Per-core fused quantize + GEMM + dequant.

    Inputs : xT [K, T] f32 (replicated, K-major), wT [K, NSH] int8,
             qs [128,1] f32 (127/(absmax+1e-8)), cs [128, NT] f32
             (act_scale * weight_scale arranged [partition, n-tile]).
    Output : out [NSH, T] f32 (the core's out-feature shard, n-major).
    """
    if "main" in _NC_CACHE:
        return _NC_CACHE["main"]
    nc = bass.Bass(name="w8a8_main")
    xT = nc.declare_dram_parameter("xT", [K, T], F32, isOutput=False)
    wT = nc.declare_dram_parameter("wT", [K, NSH], I8, isOutput=False)
    qs = nc.declare_dram_parameter("qs", [128, 1], F32, isOutput=False)
    cs = nc.declare_dram_parameter("cs", [128, NT], F32, isOutput=False)
    out = nc.declare_dram_parameter("out", [NSH, T], F32, isOutput=True)

    xT_r = xT.rearrange("(a p) t -> p a t", p=128)    # [128, KT, T]
    wT_r = wT.rearrange("(a p) n -> p a n", p=128)    # [128, KT, NSH]
    out_r = out.rearrange("(a p) t -> p a t", p=128)  # [128, NT, T]

    with tile.TileContext(nc) as tc:
        with (
            tc.tile_pool(name="const", bufs=1) as cpool,
            tc.tile_pool(name="wbf", bufs=1) as wpool,
            tc.tile_pool(name="xstage", bufs=2) as xspool,
            tc.tile_pool(name="xtmp", bufs=2) as xtpool,
            tc.tile_pool(name="xq", bufs=2) as xqpool,
            tc.tile_pool(name="psum", bufs=4, space="PSUM") as pspool,
            tc.tile_pool(name="ostage", bufs=4) as opool,
        ):
            qs_t = cpool.tile([128, 1], F32)
            nc.sync.dma_start(qs_t[:], qs[:])
            cs_t = cpool.tile([128, NT], F32)
            nc.gpsimd.dma_start(cs_t[:], cs[:])

            # PE warm-up: a few matmuls on a never-written (garbage) tile
            # keep the PE busy during the input DMAs so HAM un-throttles the
            # clock to 2.4 GHz before the first real matmul. The tile is
            # never written, so these carry no dependencies at all.
            wbf = wpool.tile([128, KT, NSH], BF16)
            warm = cpool.tile([128, TC], BF16)
            nc.gpsimd.memset(warm[:], 0.0)
            warm_ps = pspool.tile([128, TC], F32, tag="ps")
            for _ in range(14):
                nc.tensor.matmul(
                    warm_ps[:],
                    warm[:, 0:128],
                    warm[:, 0:TC],
                    start=True,
                    stop=True,
                    skip_group_check=True,
                )

            # Weights: DMA int8 -> SBUF (triggered from the GpSimd queue so
            # the Sync queue issues the latency-critical x chunk loads),
            # widen to bf16 (exact) on ACT, per k-group so the first
            # matmuls' weights are ready early. Staged in an xq-pool slot
            # (16 KiB < the 32 KiB xq slot): costs no extra SBUF.
            ws = xqpool.tile([128, KT, NSH], I8, tag="xq")
            for g in range(NQG):
                sl = slice(g * QG, (g + 1) * QG)
                nc.gpsimd.dma_start(ws[:, sl, :], wT_r[:, sl, :])
                nc.scalar.activation(
                    wbf[:, sl, :],
                    ws[:, sl, :],
                    mybir.ActivationFunctionType.Copy,
                )

            # Chunk 0 uses fine-grained quant groups so the PE pipeline
            # fills as soon as the first small x slab lands; later chunks
            # use full-size groups (fewer instructions).
            GROUPS0 = [2, 2, 4, 8, 8, 8]
            for ch in range(NCH):
                t0 = ch * TC
                # --- quantize this token chunk: xq = round(x * qs) in bf16 ---
                xq = xqpool.tile([128, KT, TC], BF16)
                groups = GROUPS0 if ch == 0 else [QG] * NQG
                k0 = 0
                for gsz in groups:
                    ksl = slice(k0, k0 + gsz)
                    k0 += gsz
                    xs = xspool.tile([128, gsz, TC], F32, tag="xs")
                    nc.sync.dma_start(xs[:], xT_r[:, ksl, t0 : t0 + TC])
                    tmp = xtpool.tile([128, gsz, TC], F32, tag="tmp")
                    # (x * qs) rounds to fp32 between the two ALU slices,
                    # then +MAGIC rounds half-even to integer: identical to
                    # jnp.round(x * scale_inv).
                    nc.vector.tensor_scalar(
                        tmp[:],
                        xs[:],
                        qs_t[:, 0:1],
                        float(MAGIC),
                        op0=mybir.AluOpType.mult,
                        op1=mybir.AluOpType.add,
                    )
                    nc.vector.tensor_scalar_add(xq[:, ksl, :], tmp[:], -float(MAGIC))
                # --- GEMM: psum[n128, TC] = sum_k wT[k,n].T @ xq[k,t] ---
                for nt in range(NT):
                    ps = pspool.tile([128, TC], F32, tag="ps")
                    for kt in range(KT):
                        nc.tensor.matmul(
                            ps[:],
                            wbf[:, kt, nt * 128 : (nt + 1) * 128],
                            xq[:, kt, :],
                            start=(kt == 0),
                            stop=(kt == KT - 1),
                        )
                    ot = opool.tile([128, TC], F32)
                    nc.scalar.activation(
                        ot[:],
                        ps[:],
                        mybir.ActivationFunctionType.Copy,
                        scale=cs_t[:, nt : nt + 1],
                    )
                    nc.sync.dma_start(out_r[:, nt, t0 : t0 + TC], ot[:])
    _split_excess_waits(nc)
    _NC_CACHE["main"] = nc
    return nc


def kernel(x, qweight, weight_scale):
    x = np.asarray(x)
    orig_dtype = x.dtype
    x2 = np.ascontiguousarray(x, dtype=np.float32).reshape(T, K)
    qw = np.asarray(qweight)
    if qw.dtype != np.int8:
        qw = qw.astype(np.int8)
    ws = np.asarray(weight_scale, dtype=np.float32)

    xT = np.ascontiguousarray(x2.T)  # [K, T]
    core_ids = list(range(NCORES))

    # --- launch 1: global absmax (each core reduces a K-shard of xT) ---
    ksh = K // NCORES
    res1 = run_bass_kernel_spmd(
        _absmax_nc(),
        [{"xs": xT[c * ksh : (c + 1) * ksh, :]} for c in core_ids],
        core_ids=core_ids,
    )
    absmax = np.float32(
        max(np.float32(r["amax"].max()) for r in res1.results)
    )
    scale_inv = np.float32(127.0) / (absmax + np.float32(1e-8))
    act_scale = absmax / np.float32(127.0)

    # --- launch 2: quantize + GEMM + dequant, column-parallel ---
    qs_arr = np.full((128, 1), scale_inv, dtype=np.float32)
    in_maps = []
    for c in core_ids:
        w_shard = qw[c * NSH : (c + 1) * NSH, :]  # [NSH, K] int8
        wT_shard = np.ascontiguousarray(w_shard.T)  # [K, NSH]
        cs_arr = np.ascontiguousarray(
            (act_scale * ws[c * NSH : (c + 1) * NSH]).reshape(NT, 128).T
        )  # [128, NT]
        in_maps.append(
            {"xT": xT, "wT": wT_shard, "qs": qs_arr, "cs": cs_arr}
        )
    res2 = run_bass_kernel_spmd(_main_nc(), in_maps, core_ids=core_ids)

    outT = np.concatenate(
        [res2.results[c]["out"] for c in core_ids], axis=0
    )  # [N, T]
    return (
        np.ascontiguousarray(outT.T)
        .reshape(B, S, N)
        .astype(orig_dtype, copy=False)
    )
